# revision 11
# baseline (speedup 1.0000x reference)
"""Trainium2 Bass kernel for InterpBaselineEncoder (histogram binning), v2.

See reference: coarsen 128x128 grid 4x4 -> 1024 cells; scatter-mean U=8192
off-grid points (+ on-grid cell values) into cells via closed-form binning
round_ne(p*127/4 - 0.375); gather cell averages at T targets.

Bin = 32i+j split as hi = 4i + j//8 (128, PSUM partitions) and lo = j%8.
Scatter: psum[hi, (lo,y')] += oh128(hi)[u] * (oh8(lo)[u] * [y,1][u]); the
ones column produces counts.  On-grid cells enter as 8 pseudo-point tiles
with host-precomputed constant one-hots.  Gather: broadcast target hi rows
by selector matmul, one-hot on ACT (relu(1-(x-q)^2) of integer distance),
gather avg rows by matmul, contract the lo one-hot on DVE.

One-hot construction runs on DVE (is_equal vs iota rows) or optionally on
GpSimd via the local_scatter ucode (per-partition scatter of ones/values
at computed int16 offsets) to offload the Vector engine.

Sharding: 8 cores = 4 batches x 2 target halves; SPMD, per-core inputs.
"""
import sys
import numpy as np

for _p in ("/opt/trn_rl_repo", "/opt/pypackages"):
    if _p not in sys.path:
        sys.path.insert(0, _p)

import ml_dtypes  # noqa: E402
from concourse import bass, bacc, mybir, tile  # noqa: E402
from concourse.bass_utils import run_bass_kernel_spmd  # noqa: E402
from concourse.bass import _add_dep_helper  # noqa: E402

F32 = mybir.dt.float32
BF16 = mybir.dt.bfloat16
I16 = mybir.dt.int16
ALU = mybir.AluOpType
ACTF = mybir.ActivationFunctionType

B, U, T, Y = 4, 8192, 4096, 8
TH = T // 2            # targets per core (2048)
KT = U // 128          # 64 point tiles
NT = TH // 128         # 16 target tiles
HI, LO = 128, 8        # bin split: bin = 32i + j = 8*hi + lo
CH = 16                # point tiles per one-hot chunk
NG = NT // 4           # gather groups of 4 tiles

RA_LS = True           # build ra via gpsimd local_scatter
W2_LS = False          # build w2 via gpsimd local_scatter
_RA_CALLS = (14, 14, 14, 14)
_RA_DVE = 8            # trailing ra tiles built on DVE
_W2_CALLS = (22, 22, 20)

# closed-form bin constants: centers c_k = (4k+1.5)/127, step 4/127
_INV = 127.0 / 4.0
_OFF0 = float(np.float32(-(1.5 / 127.0) * _INV))
_MAGIC = 12582912.0  # 1.5*2^23: (z+M)-M rounds to nearest-even integer
# (1.5*2^23 keeps z+M in the unit-spacing zone [2^23, 2^24) even for z<0)

# f32 const block [128, cols]
_CF_COLS = 1 + 1 + 128 + 8 + 128 + KT + KT + KT * 9
# bf16 const block [128, cols]: raps(8*128) blps(8*8) pmat(32) ones(KT)
_CB_COLS = 8 * 128 + 8 * 8 + 32 + KT
# f32 input block [128, 160]: py xty px xtx
_IN_COLS = KT + NT + KT + NT


def _emit_bin(nc, pool, p_ap, n, nm):
    """clamp(round_ne(p*INV+OFF0), 0, 31) -> [128, n] f32 (3 vector ops)."""
    z = pool.tile([128, n], F32, tag=f"binz{nm}")
    idx = pool.tile([128, n], F32, tag=f"bini{nm}")
    nc.vector.tensor_scalar(z[:], p_ap, _INV, _OFF0, ALU.mult, ALU.add)
    nc.vector.tensor_scalar(idx[:], z[:], _MAGIC, _MAGIC, ALU.add, ALU.subtract)
    out = pool.tile([128, n], F32, tag=f"binc{nm}")
    nc.vector.tensor_scalar(out[:], idx[:], 0.0, 31.0, ALU.max, ALU.min)
    return out


def _emit_hilo(nc, pool, iv, jv, n, nm):
    """From i,j in [0,32) compute hi = 4i + j//8 and lo = j%8 (f32)."""
    t1 = pool.tile([128, n], F32, tag=f"t1{nm}")
    jh = pool.tile([128, n], F32, tag=f"jh{nm}")
    jh8 = pool.tile([128, n], F32, tag=f"jh8{nm}")
    lo = pool.tile([128, n], F32, tag=f"lo{nm}")
    i4 = pool.tile([128, n], F32, tag=f"i4{nm}")
    hi = pool.tile([128, n], F32, tag=f"hi{nm}")
    nc.vector.tensor_scalar(t1[:], jv[:], 0.125, -0.4999, ALU.mult, ALU.add)
    nc.vector.tensor_scalar(jh[:], t1[:], _MAGIC, _MAGIC, ALU.add, ALU.subtract)
    nc.vector.tensor_scalar(jh8[:], jh[:], 8.0, None, ALU.mult)
    nc.vector.tensor_tensor(lo[:], jv[:], jh8[:], ALU.subtract)
    nc.vector.tensor_scalar(i4[:], iv[:], 4.0, None, ALU.mult)
    nc.vector.tensor_tensor(hi[:], i4[:], jh[:], ALU.add)
    return hi, lo


def build_nc():
    nc = bacc.Bacc("TRN2", target_bir_lowering=False, debug=False)

    constF = nc.declare_dram_parameter("constF", [128, _CF_COLS], F32,
                                       isOutput=False)
    constB = nc.declare_dram_parameter("constB", [128, _CB_COLS], BF16,
                                       isOutput=False)
    selB = nc.declare_dram_parameter("selB", [16, NT * 128], BF16,
                                     isOutput=False)
    inF = nc.declare_dram_parameter("inF", [128, _IN_COLS], F32,
                                    isOutput=False)
    ybfD = nc.declare_dram_parameter("ybf", [128, KT * 9], BF16,
                                     isOutput=False)
    ycON = nc.declare_dram_parameter("ycON", [128, 1024], BF16, isOutput=False)
    out_d = nc.declare_dram_parameter("out", [TH, Y], F32, isOutput=True)

    with tile.TileContext(nc) as tc:
        with (
            tc.tile_pool(name="const", bufs=1) as cpool,
            tc.tile_pool(name="work", bufs=1) as wpool,
            tc.tile_pool(name="psS", bufs=1, space="PSUM") as psS,
            tc.tile_pool(name="psP", bufs=1, space="PSUM") as psP,
            tc.tile_pool(name="psB", bufs=2, space="PSUM") as psB,
            tc.tile_pool(name="psR", bufs=1, space="PSUM") as psR,
        ):
            # ---- input DMAs, split across the two HWDGE queues ----
            tin = wpool.tile([128, _IN_COLS], F32, tag="tin")
            nc.sync.dma_start(tin[:], inF[:])
            cb = cpool.tile([128, _CB_COLS], BF16, tag="cb")
            nc.scalar.dma_start(cb[:], constB[:])
            t_ycon = wpool.tile([128, 1024], BF16, tag="ycon")
            nc.sync.dma_start(t_ycon[:], ycON[:])
            cf = cpool.tile([128, _CF_COLS], F32, tag="cf")
            nc.scalar.dma_start(cf[:], constF[:])
            c_selB = cpool.tile([16, NT * 128], BF16, tag="selB")
            nc.sync.dma_start(c_selB[:], selB[:])
            t_ybf = wpool.tile([128, KT, 9], BF16, tag="ybf")
            nc.scalar.dma_start(
                t_ybf[:], ybfD[:].rearrange("p (k y) -> p k y", y=9))

            o = 0
            c_iotaP = cf[:, o:o + 1]; o += 1          # [128,1] p
            c_niotaP = cf[:, o:o + 1]; o += 1         # [128,1] -p
            c_i128row = cf[:, o:o + 128]; o += 128    # rows 0..127
            c_i8row = cf[:, o:o + 8]; o += 8          # rows 0..7
            c_ident = cf[:, o:o + 128]; o += 128
            c_rabase = cf[:, o:o + KT]; o += KT       # 128*(k - call_start)
            c_ohbase = cf[:, o:o + KT]; o += KT       # 8*k
            c_w2base = cf[:, o:o + KT * 9]            # 72*(k-start) + y
            c_w2base = c_w2base.rearrange("p (k y) -> p k y", y=9)
            c_raps = cb[:, 0:1024].rearrange("p (m q) -> p m q", q=128)
            c_blps = cb[:, 1024:1088].rearrange("p (m l) -> p m l", l=8)
            c_pmat = cb[:, 1088:1120]
            c_onesK = cb[:, 1120:1120 + KT]
            c_sel = c_selB[:].rearrange("p (n q) -> p n q", q=128)

            NB = KT + NT  # 80: off-grid then target coords, fused binning
            t_yc = tin[:, 0:NB]
            t_xc = tin[:, NB:2 * NB]

            # ---- fused off-grid + target binning (DVE) ----
            ia = _emit_bin(nc, wpool, t_yc, NB, "a")
            ja = _emit_bin(nc, wpool, t_xc, NB, "a2")
            hia, loa = _emit_hilo(nc, wpool, ia, ja, NB, "a")
            hio, loo = hia[:, 0:KT], loa[:, 0:KT]
            hit, lot = hia[:, KT:NB], loa[:, KT:NB]

            # ra index build first: it gates the gpsimd local_scatter chain
            KG = KT - _RA_DVE
            rai = wpool.tile([128, KG], I16, tag="rai")
            i_rai = nc.vector.tensor_tensor(rai[:], hio[:, 0:KG],
                                            c_rabase[:, 0:KG], ALU.add)

            # ---- lo one-hots ----
            oh8t = wpool.tile([128, NT, LO], BF16, tag="oh8t")
            if not W2_LS:
                oh8 = wpool.tile([128, KT, LO], BF16, tag="oh8")
                nc.vector.tensor_tensor(
                    oh8[:],
                    c_i8row.unsqueeze(1).broadcast_to((128, KT, LO)),
                    loo.unsqueeze(2).broadcast_to((128, KT, LO)),
                    ALU.is_equal,
                )
            nc.vector.tensor_tensor(
                oh8t[:],
                c_i8row.unsqueeze(1).broadcast_to((128, NT, LO)),
                lot.unsqueeze(2).broadcast_to((128, NT, LO)),
                ALU.is_equal,
            )

            # ---- pooling: 4 accumulating matmuls over w-phases ----
            yv = t_ycon[:].rearrange("p (w c y) -> p w c y", c=4, y=Y)
            pp = psP.tile([32, 32, Y], F32, tag="pp")
            for c in range(4):
                nc.tensor.matmul(pp[:], c_pmat, yv[:, :, c, :],
                                 start=(c == 0), stop=(c == 3))
            gvabf = wpool.tile([32, 32, 9], BF16, tag="gvabf")
            nc.vector.memset(gvabf[:, :, 8:9], 1.0)
            nc.scalar.copy(gvabf[:, :, 0:8], pp[:])
            ypsb = wpool.tile([128, 8, 9], BF16, tag="ypsb")
            nc.sync.dma_start(ypsb[:], gvabf[:])

            # pseudo-point moving operand: w2ps = blps (const) x ypsb
            w2ps = wpool.tile([128, 8, LO, 9], BF16, tag="w2ps")
            nc.vector.tensor_tensor(
                w2ps[:],
                c_blps.unsqueeze(3).broadcast_to((128, 8, LO, 9)),
                ypsb[:].unsqueeze(2).broadcast_to((128, 8, LO, 9)),
                ALU.mult,
            )

            # ---- target transpose + broadcast + hi one-hot (ACT) ----
            pst = psP.tile([16, 128], F32, tag="pst")
            nc.tensor.transpose(pst[:], hit, c_ident)
            ihjTbf = wpool.tile([16, 128], BF16, tag="ihjTbf")
            nc.scalar.copy(ihjTbf[:], pst[:])

            rt4s = []
            for g in range(NG):
                pb4 = psB.tile([128, 4, 128], F32, tag="pb4")
                for m in range(4):
                    nc.tensor.matmul(pb4[:, m, :], c_sel[:, 4 * g + m, :],
                                     ihjTbf[:], start=True, stop=True)
                sq4 = wpool.tile([128, 4 * 128], F32, tag="sq4")
                nc.scalar.activation(sq4[:], pb4[:].rearrange("p m q -> p (m q)"),
                                     ACTF.Square, bias=c_niotaP, scale=1.0)
                rt4 = wpool.tile([128, 4, 128], BF16, tag=f"rt4_{g}")
                nc.scalar.activation(rt4[:].rearrange("p m q -> p (m q)"),
                                     sq4[:], ACTF.Relu, bias=1.0, scale=-1.0)
                rt4s.append(rt4)

            # ---- scatter one-hots ----
            ra = wpool.tile([128, KT, HI], BF16, tag="ra")
            w2 = wpool.tile([128, KT, LO, 9], BF16, tag="w2")
            s = 0
            for ntile in _RA_CALLS:
                nc.gpsimd.local_scatter(
                    ra[:, s:s + ntile, :].rearrange("p k q -> p (k q)"),
                    c_onesK[:, s:s + ntile],
                    rai[:, s:s + ntile],
                    channels=128, num_elems=ntile * HI, num_idxs=ntile)
                s += ntile
            i_radve = nc.vector.tensor_tensor(
                ra[:, KG:KT, :],
                c_i128row.unsqueeze(1).broadcast_to((128, _RA_DVE, HI)),
                hio[:, KG:KT].unsqueeze(2).broadcast_to((128, _RA_DVE, HI)),
                ALU.is_equal,
            )
            if W2_LS:
                lo9 = wpool.tile([128, KT], F32, tag="lo9")
                w2bs = wpool.tile([128, KT, 9], F32, tag="w2bs")
                w2i = wpool.tile([128, KT, 9], I16, tag="w2i")
                nc.vector.tensor_scalar(lo9[:], loo, 9.0, None, ALU.mult)
                nc.vector.tensor_tensor(
                    w2bs[:], c_w2base,
                    lo9[:].unsqueeze(2).broadcast_to((128, KT, 9)), ALU.add)
                nc.vector.tensor_copy(w2i[:], w2bs[:])
                s = 0
                for ntile in _W2_CALLS:
                    nc.gpsimd.local_scatter(
                        w2[:, s:s + ntile].rearrange("p k l y -> p (k l y)"),
                        t_ybf[:, s:s + ntile, :].rearrange("p k y -> p (k y)"),
                        w2i[:, s:s + ntile, :].rearrange("p k y -> p (k y)"),
                        channels=128, num_elems=ntile * LO * 9,
                        num_idxs=ntile * 9)
                    s += ntile
            else:
                for c0 in range(0, KT, CH):
                    sl = slice(c0, c0 + CH)
                    i_w2 = nc.vector.tensor_tensor(
                        w2[:, sl, :, :],
                        oh8[:, sl, :].unsqueeze(3).broadcast_to((128, CH, LO, 9)),
                        t_ybf[:, sl, :].unsqueeze(2).broadcast_to((128, CH, LO, 9)),
                        ALU.mult,
                    )
                    if c0 == 0:
                        _add_dep_helper(i_w2.ins, i_rai.ins, sync=False,
                                        reason="rai gates gpsimd; run it first")
                    _add_dep_helper(i_radve.ins, i_w2.ins, sync=False,
                                    reason="dve ra tail after w2 chunks")

            # ---- scatter matmul stream, ordered by producer readiness ----
            ps = psS.tile([128, LO * 9], F32, tag="ps")
            for m in range(8):
                nc.tensor.matmul(ps[:], c_raps[:, m, :], w2ps[:, m, :, :],
                                 start=(m == 0), stop=False)
            k_last = KG - 1
            order = [*range(0, KG - _RA_CALLS[-1]), *range(KG, KT),
                     *range(KG - _RA_CALLS[-1], KG)]
            for k in order:
                nc.tensor.matmul(ps[:], ra[:, k, :], w2[:, k, :, :],
                                 start=False, stop=(k == k_last))

            # ---- per-bin averages: avgM[128, (y, lo)] bf16 ----
            psv = ps[:].rearrange("p (l y) -> p l y", y=9)
            rc = wpool.tile([128, LO], F32, tag="rc")
            nc.vector.reciprocal(rc[:], psv[:, :, 8])
            avgM = wpool.tile([128, Y, LO], BF16, tag="avgM")
            nc.vector.tensor_tensor(
                avgM[:],
                psv[:, :, 0:8].transpose([0, 2, 1]),
                rc[:].unsqueeze(1).broadcast_to((128, Y, LO)),
                ALU.mult,
            )

            # ---- gather matmuls + lo contraction, two pipelined halves ----
            outsb = wpool.tile([128, NT, Y], F32, tag="outsb")
            H = NT // 2
            for h in range(2):
                rv = psR.tile([128, H, Y, LO], F32, tag=f"rv{h}")
                for j in range(H):
                    n = h * H + j
                    nc.tensor.matmul(
                        rv[:, j, :, :], rt4s[n // 4][:, n % 4, :],
                        avgM[:].rearrange("p y l -> p (y l)"),
                        start=True, stop=True)
                tmp = wpool.tile([128, H, Y, LO], F32, tag=f"tmp{h}")
                nc.vector.tensor_tensor(
                    tmp[:],
                    rv[:],
                    oh8t[:, h * H:(h + 1) * H, :].unsqueeze(2)
                        .broadcast_to((128, H, Y, LO)),
                    ALU.mult,
                )
                nc.vector.tensor_reduce(outsb[:, h * H:(h + 1) * H, :], tmp[:],
                                        axis=mybir.AxisListType.X, op=ALU.add)

            nc.sync.dma_start(
                out_d[:].rearrange("(p n) y -> p (n y)", p=128), outsb[:])
    nc.compile()
    return nc


def _consts():
    cf = np.zeros((128, _CF_COLS), np.float32)
    o = 0
    cf[:, o] = np.arange(128, dtype=np.float32); o += 1
    cf[:, o] = -np.arange(128, dtype=np.float32); o += 1
    cf[:, o:o + 128] = np.arange(128, dtype=np.float32)[None, :]; o += 128
    cf[:, o:o + 8] = np.arange(8, dtype=np.float32)[None, :]; o += 8
    cf[:, o:o + 128] = np.eye(128, dtype=np.float32); o += 128
    rabase = np.zeros(KT, np.float32)
    s = 0
    for ntile in _RA_CALLS:
        rabase[s:s + ntile] = 128.0 * np.arange(ntile)
        s += ntile
    assert s == KT - _RA_DVE
    cf[:, o:o + KT] = rabase[None, :]; o += KT
    cf[:, o:o + KT] = 8.0 * np.arange(KT, dtype=np.float32)[None, :]; o += KT
    w2base = np.zeros((KT, 9), np.float32)
    s = 0
    for ntile in _W2_CALLS:
        w2base[s:s + ntile] = (72.0 * np.arange(ntile)[:, None]
                               + np.arange(9)[None, :])
        s += ntile
    cf[:, o:o + KT * 9] = w2base.reshape(1, KT * 9)

    s = 8 * np.arange(128)[:, None] + np.arange(8)[None, :]  # [128, 8]
    si, sj = s // 32, s % 32
    hi_ps = 4 * si + sj // 8          # [128, 8] in [0,128)
    lo_ps = sj % 8
    raps = (np.arange(128)[None, None, :] == hi_ps[:, :, None])
    blps = (np.arange(8)[None, None, :] == lo_ps[:, :, None])
    pmat = np.zeros((128, 32), np.float32)
    for h in range(128):
        pmat[h, h // 4] = 1.0 / 16.0
    cb = np.zeros((128, _CB_COLS), np.float32)
    cb[:, 0:1024] = raps.reshape(128, 1024)
    cb[:, 1024:1088] = blps.reshape(128, 64)
    cb[:, 1088:1120] = pmat
    cb[:, 1120:1120 + KT] = 1.0

    sel = (np.arange(16)[:, None] == np.arange(NT)[None, :])  # [16, NT]
    selb = np.repeat(sel[:, :, None], 128, axis=2).reshape(16, NT * 128)
    return {
        "constF": cf,
        "constB": cb.astype(ml_dtypes.bfloat16),
        "selB": selb.astype(ml_dtypes.bfloat16),
    }


def _stage_core(xc_off, yc_off, yc_on, xt, b, half):
    m = {}
    fin = np.empty((128, _IN_COLS), np.float32)
    sl = slice(half * TH, (half + 1) * TH)
    o = 0
    fin[:, o:o + KT] = xc_off[b, :, 0].reshape(KT, 128).T; o += KT
    # target (p, n) holds xt row p*16+n so the output DMA is contiguous
    fin[:, o:o + NT] = xt[b, sl, 0].reshape(128, NT); o += NT
    fin[:, o:o + KT] = xc_off[b, :, 1].reshape(KT, 128).T; o += KT
    fin[:, o:o + NT] = xt[b, sl, 1].reshape(128, NT); o += NT
    m["inF"] = fin
    ybf = np.ones((128, KT, 9), np.float32)
    ybf[:, :, 0:8] = yc_off[b].reshape(KT, 128, Y).transpose(1, 0, 2)
    m["ybf"] = ybf.reshape(128, KT * 9).astype(ml_dtypes.bfloat16)
    m["ycON"] = np.ascontiguousarray(yc_on[b].reshape(128, 1024)).astype(
        ml_dtypes.bfloat16)
    return m


def _in_maps(inputs):
    xc_off_grid = np.ascontiguousarray(inputs["xc_off_grid"], np.float32)
    yc_off_grid = np.ascontiguousarray(inputs["yc_off_grid"], np.float32)
    yc_on_grid = np.ascontiguousarray(inputs["yc_on_grid"], np.float32)
    xt = np.ascontiguousarray(inputs["xt"], np.float32)
    consts = _consts()
    in_maps = []
    for core in range(8):
        b, half = core // 2, core % 2
        m = dict(consts)
        m.update(_stage_core(xc_off_grid, yc_off_grid, yc_on_grid, xt, b, half))
        in_maps.append(m)
    return in_maps


_NC = None


def kernel(xc_off_grid, yc_off_grid, xc_on_grid, yc_on_grid, xt):
    global _NC
    if _NC is None:
        _NC = build_nc()
    nc = _NC

    in_maps = _in_maps(dict(xc_off_grid=xc_off_grid, yc_off_grid=yc_off_grid,
                            yc_on_grid=yc_on_grid, xt=xt))

    res = run_bass_kernel_spmd(nc, in_maps, list(range(8)))
    out = np.empty((B, T, Y), np.float32)
    for core in range(8):
        b, half = core // 2, core % 2
        out[b, half * TH:(half + 1) * TH] = res.results[core]["out"]
    return out
